# revision 1
# baseline (speedup 1.0000x reference)
"""Decorrelated (whitening) group norm for Trainium2, 8 NeuronCores.

Problem: x (16, 64, 224, 224) f32; G=32 groups where group(channel-row r) = r % 32
(after flattening batch*channel to 1024 rows). Whitening: y = sigma^{-1/2} (x - mean)
per group, sigma the 32x32 group covariance.

Strategy (single NEFF, SPMD on 8 cores, data-parallel over batch):
  - Core k gets rows [128k, 128k+128) as a (128, 50176) f32 tensor; row p is group
    p % 32 (4 group-blocks of 32 on the partition axis). Whole shard stays resident
    in SBUF (~196 KiB/partition).
  - Pass 1: for each 128-col chunk: PE-transpose (fp32) -> PSUM, copy-cast to bf16
    into a (128,129) staging tile whose last column is a constant 1.0, then one PE
    matmul accumulates [gram | row-sums] = T^T [T | 1] into a persistent PSUM bank.
  - Block-reduce gram/sums to (32, 33) with tiny selection matmuls; AllReduce the
    4 KB partial over the 8 cores (TOPSP collective).
  - On-device 32x32 math: sigma = S/n - mean mean^T + eps I; Newton-Schulz
    iterations give wm = sigma^{-1/2} (equals the reference's SVD-based result to
    fp32 precision since sigma's eigenvalues are ~1).
  - Pass 2: y = x + bdiag(wm - I) @ x - wm@mean. The correction matmul runs in
    float32r (full PE rate) on an ACT-side rounded copy of the chunk; the exact +x
    and per-partition bias are fused into one DVE scalar_tensor_tensor during PSUM
    eviction; DMA out. (x itself is never rounded - the f32r data is a scratch
    copy, so the identity path stays exact.)
"""

import functools
import os
import sys

import numpy as np

if "/opt/trn_rl_repo" not in sys.path:
    sys.path.insert(0, "/opt/trn_rl_repo")

B, C, H, W = 16, 64, 224, 224
G = 32
EPS = 1e-5
NCORES = 8
ROWS = 128                 # per-core rows = 2 batches * 64 channels
COLS = H * W               # 50176
NS_ITERS = 4

LAST_RESULTS = None        # BassKernelResults of the most recent run (for test harness)


@functools.lru_cache(maxsize=4)
def _build(cols, ncores, f32r_apply=True, cast_engine="scalar", lookahead=2):
    import concourse.bass as bass
    import concourse.tile as tile
    from concourse import bacc, mybir

    f32 = mybir.dt.float32
    f32r = mybir.dt.float32r
    bf16 = mybir.dt.bfloat16
    ADD = mybir.AluOpType.add

    ntot = 4 * cols * ncores          # elements per group, all cores
    ntch = cols // 128                # transpose chunks
    nach = cols // 512                # apply chunks
    nlch = 16                         # load DMAs
    assert cols % 1024 == 0 and cols % 128 == 0 and cols % nlch == 0

    nc = bacc.Bacc(
        "TRN2", target_bir_lowering=False, debug=False, num_devices=ncores
    )
    xin = nc.dram_tensor("x", [ROWS, cols], f32, kind="ExternalInput")
    yout = nc.dram_tensor("y", [ROWS, cols], f32, kind="ExternalOutput")
    xin_ap = xin.ap()
    yout_ap = yout.ap()

    import ml_dtypes
    i128_d = nc.inline_tensor(np.eye(128, dtype=np.float32), name="i128c")
    i128b_d = nc.inline_tensor(
        np.eye(128).astype(ml_dtypes.bfloat16), name="i128bc"
    )
    p4_d = nc.inline_tensor(
        np.tile(np.eye(32, dtype=np.float32), (4, 1)), name="p4c"
    )


    with tile.TileContext(nc) as tc:
        with (
            tc.tile_pool(name="xpool", bufs=1) as xpool,
            tc.tile_pool(name="smalls", bufs=1) as smalls,
        ):
          with tc.tile_pool(name="consts", bufs=1) as consts:
            # ---- constants ----
            i128 = consts.tile([128, 128], f32, name="i128")
            nc.sync.dma_start(i128[:], i128_d.ap())
            i128b = consts.tile([128, 128], bf16, name="i128b")
            nc.sync.dma_start(i128b[:], i128b_d.ap())
            p4 = consts.tile([128, 32], f32, name="p4")
            nc.sync.dma_start(p4[:], p4_d.ap())
            ones32f = consts.tile([32, 1], f32, name="ones32f")
            nc.vector.memset(ones32f[:], 1.0)
            onesrow = consts.tile([1, 32], f32, name="onesrow")
            nc.vector.memset(onesrow[:], 1.0)
            c15I = consts.tile([32, 32], f32, name="c15I")
            nc.vector.tensor_scalar_mul(c15I[:], i128[0:32, 0:32], 1.5)
            epsI = consts.tile([32, 32], f32, name="epsI")
            nc.vector.tensor_scalar_mul(epsI[:], i128[0:32, 0:32], EPS)

            # warmup AllReduce: absorbs ncfw startup latency while pass 1
            # runs; collectives execute on TOPSP, not the compute engines.
            with tc.tile_pool(name="dramw", bufs=1, space="DRAM") as dramw:
                win = dramw.tile([32, 1], f32, name="cc_win")
                wout = dramw.tile([32, 1], f32, name="cc_wout")
                nc.sync.dma_start(win[:], ones32f[:])
                nc.gpsimd.collective_compute(
                    "AllReduce",
                    mybir.AluOpType.add,
                    replica_groups=[list(range(ncores))],
                    ins=[win.opt()],
                    outs=[wout.opt()],
                )

            # ---- resident shard load ----
            xres = xpool.tile([128, cols], f32, name="xres")
            lch = cols // nlch
            # front chunks are small so compute can start sooner
            bounds = [0]
            for w in ([lch // 4, lch // 4, lch // 2] + [lch] * (nlch - 1)):
                bounds.append(bounds[-1] + w)
            assert bounds[-1] == cols
            for k in range(len(bounds) - 1):
                nc.sync.dma_start(
                    xres[:, bounds[k]:bounds[k + 1]],
                    xin_ap[:, bounds[k]:bounds[k + 1]],
                )

            # HAM warmup: a ~4us burst of real matmuls un-throttles the PE
            # clock (transpose-mode ops alone never register as PE-busy).
            with (
                tc.tile_pool(name="warmb", bufs=1) as warmb,
                tc.tile_pool(name="psW", bufs=1, space="PSUM") as psW,
            ):
                wsrc = warmb.tile([128, 128], bf16, name="wsrc")
                nc.vector.memset(wsrc[:], 0.0)
                wps = psW.tile([128, 128], f32, name="wps")
                nwarm = 40
                for i in range(nwarm):
                    nc.tensor.matmul(
                        wps[:], wsrc[:], wsrc[:],
                        start=(i == 0), stop=(i == nwarm - 1),
                    )
                wev = warmb.tile([128, 128], bf16, name="wev")
                nc.vector.tensor_copy(wev[:], wps[:])

            # ---- pass 1: [gram | sums] accumulation ----
            with (
                tc.tile_pool(name="psA", bufs=4, space="PSUM") as psA,
                tc.tile_pool(name="psAcc", bufs=1, space="PSUM") as psAcc,
                tc.tile_pool(name="tstage", bufs=1) as tstage,
                tc.tile_pool(name="xbst", bufs=4) as xbst,
            ):
                gramP = psAcc.tile([128, 129], f32, name="gramP")

                # persistent bf16 staging tiles; col 128 stays 1.0 so one
                # matmul accumulates both the gram and the row sums.
                nbuf = 4
                tbs = []
                for b in range(nbuf):
                    tb = tstage.tile([128, 129], bf16, name=f"tb{b}")
                    nc.vector.memset(tb[:, 128:129], 1.0)
                    tbs.append(tb)

                def emit_transpose(j):
                    # cheap 4x-mode DVE cast, then everything stays bf16:
                    # bf16 LDWEIGHTS gets FWL and bf16 transpose runs 1 cy/row
                    xb = xbst.tile([128, 128], bf16, name="xb")
                    nc.vector.tensor_copy(xb[:], xres[:, j * 128:(j + 1) * 128])
                    tp = psA.tile([128, 128], bf16, name="tp")
                    nc.tensor.transpose(tp[:], xb[:], i128b[:])
                    tb = tbs[j % nbuf]
                    if j % 2 == 0:
                        nc.vector.tensor_copy(tb[:, 0:128], tp[:])
                    else:
                        nc.scalar.copy(tb[:, 0:128], tp[:])

                def emit_cov(j):
                    tb = tbs[j % nbuf]
                    nc.tensor.matmul(
                        gramP[:], tb[:, 0:128], tb[:],
                        start=(j == 0), stop=(j == ntch - 1),
                    )

                la = min(lookahead, ntch)
                for j in range(ntch):
                    emit_transpose(j)
                    if j >= la:
                        emit_cov(j - la)
                for j in range(ntch - la, ntch):
                    emit_cov(j)

                # evict accumulator while its bank is still in scope
                gram_sb = smalls.tile([128, 129], f32, name="gram_sb")
                nc.vector.tensor_copy(gram_sb[:], gramP[:])

            # ---- block reduce to (32, 33) + allreduce + 32x32 math ----
            # mid-phase-only tiles live in their own pool so pass 2 gets the
            # SBUF back.
            with (
                tc.tile_pool(name="psS", bufs=3, space="PSUM") as psS,
                tc.tile_pool(name="mids", bufs=1) as mids,
            ):
                sigP = psS.tile([32, 32], f32, name="psml")
                for i in range(4):
                    nc.tensor.matmul(
                        sigP[:],
                        i128[:, 32 * i:32 * (i + 1)],
                        gram_sb[:, 32 * i:32 * (i + 1)],
                        start=(i == 0), stop=(i == 3),
                    )
                s1rP = psS.tile([32, 1], f32, name="psml")
                nc.tensor.matmul(
                    s1rP[:], p4[:], gram_sb[:, 128:129], start=True, stop=True
                )

                part = mids.tile([32, 33], f32, name="part")
                nc.vector.tensor_copy(part[:, 0:32], sigP[:])
                nc.vector.tensor_copy(part[:, 32:33], s1rP[:])

                with tc.tile_pool(name="dram", bufs=1, space="DRAM") as dram:
                    cin = dram.tile([32, 33], f32, name="cc_in")
                    cout = dram.tile([32, 33], f32, name="cc_out")
                    nc.sync.dma_start(cin[:], part[:])
                    nc.gpsimd.collective_compute(
                        "AllReduce",
                        mybir.AluOpType.add,
                        replica_groups=[list(range(ncores))],
                        ins=[cin.opt()],
                        outs=[cout.opt()],
                    )
                    ar = mids.tile([32, 33], f32, name="ar")
                    nc.sync.dma_start(ar[:], cout[:])

                # ---- tiny stats math ----
                inv_n = 1.0 / float(ntot)
                mean = mids.tile([32, 1], f32, name="mean")
                nc.vector.tensor_scalar_mul(mean[:], ar[:, 32:33], inv_n)
                sig0 = mids.tile([32, 32], f32, name="sig0")
                nc.vector.tensor_scalar_mul(sig0[:], ar[:, 0:32], inv_n)

                # mean row (1,32) via PE transpose
                mrowP = psS.tile([1, 32], f32, name="psml")
                nc.tensor.transpose(mrowP[:], mean[:], i128[0:32, 0:32])
                mrow = mids.tile([1, 32], f32, name="mrow")
                nc.vector.tensor_copy(mrow[:], mrowP[:])
                outerP = psS.tile([32, 32], f32, name="psml")
                nc.tensor.matmul(outerP[:], mrow[:], mrow[:], start=True, stop=True)

                sigma = mids.tile([32, 32], f32, name="sigma")
                nc.vector.scalar_tensor_tensor(
                    out=sigma[:], in0=outerP[:], scalar=-1.0, in1=sig0[:],
                    op0=mybir.AluOpType.mult, op1=ADD,
                )
                nc.vector.tensor_add(sigma[:], sigma[:], epsI[:])

                # t = trace(sigma)/32, per-partition scalars rt = 1/t, rs = t^-1/2
                diag = mids.tile([32, 32], f32, name="diag")
                nc.vector.tensor_mul(diag[:], sigma[:], i128[0:32, 0:32])
                dvec = mids.tile([32, 1], f32, name="dvec")
                nc.vector.reduce_sum(dvec[:], diag[:], axis=mybir.AxisListType.X)
                trP = psS.tile([1, 1], f32, name="psml")
                nc.tensor.matmul(trP[:], dvec[:], ones32f[:], start=True, stop=True)
                tr_sb = mids.tile([1, 1], f32, name="tr_sb")
                nc.vector.tensor_copy(tr_sb[:], trP[:])
                tr4P = psS.tile([32, 1], f32, name="psml")
                nc.tensor.matmul(tr4P[:], onesrow[:], tr_sb[:], start=True, stop=True)
                tA = mids.tile([32, 1], f32, name="tA")
                nc.vector.tensor_scalar_mul(tA[:], tr4P[:], 1.0 / 32.0)
                rt = mids.tile([32, 1], f32, name="rt")
                nc.vector.reciprocal(rt[:], tA[:])
                rs = mids.tile([32, 1], f32, name="rs")
                nc.scalar.activation(
                    rs[:], rt[:], mybir.ActivationFunctionType.Sqrt
                )

                A = mids.tile([32, 32], f32, name="A")
                nc.vector.tensor_scalar_mul(A[:], sigma[:], rt[:])

                # ---- Newton-Schulz: Y0 = A, Z0 = I ----
                Y = mids.tile([32, 32], f32, name="Y")
                nc.vector.tensor_copy(Y[:], A[:])
                Z = mids.tile([32, 32], f32, name="Z")
                nc.vector.tensor_copy(Z[:], i128[0:32, 0:32])
                with tc.tile_pool(name="nsbuf", bufs=2) as nsbuf:
                    for _ in range(NS_ITERS):
                        zyP = psS.tile([32, 32], f32, name="psml")
                        nc.tensor.matmul(zyP[:], Z[:], Y[:], start=True, stop=True)
                        Wt = nsbuf.tile([32, 32], f32, name="Wt")
                        nc.vector.scalar_tensor_tensor(
                            out=Wt[:], in0=zyP[:], scalar=-0.5, in1=c15I[:],
                            op0=mybir.AluOpType.mult, op1=ADD,
                        )
                        ypP = psS.tile([32, 32], f32, name="psml")
                        nc.tensor.matmul(ypP[:], Y[:], Wt[:], start=True, stop=True)
                        zpP = psS.tile([32, 32], f32, name="psml")
                        nc.tensor.matmul(zpP[:], Wt[:], Z[:], start=True, stop=True)
                        nc.vector.tensor_copy(Y[:], ypP[:])
                        nc.vector.tensor_copy(Z[:], zpP[:])

                # wm = Z / sqrt(t); R = wm - I; negb = -wm @ mean
                wm = mids.tile([32, 32], f32, name="wm")
                nc.vector.tensor_scalar_mul(wm[:], Z[:], rs[:])
                R = mids.tile([32, 32], f32, name="R")
                nc.vector.tensor_sub(R[:], wm[:], i128[0:32, 0:32])
                bP = psS.tile([32, 1], f32, name="psml")
                nc.tensor.matmul(bP[:], wm[:], mean[:], start=True, stop=True)
                negb = mids.tile([32, 1], f32, name="negb")
                nc.vector.tensor_scalar_mul(negb[:], bP[:], -1.0)

                # broadcast to 128 partitions: b4, WM4R = bdiag(R)
                b4 = smalls.tile([128, 1], f32, name="b4")
                wm4r_f = mids.tile([128, 128], f32, name="wm4r_f")
                nc.vector.memset(wm4r_f[:], 0.0)
                for i in range(4):
                    eng = nc.sync if i % 2 == 0 else nc.scalar
                    eng.dma_start(b4[32 * i:32 * (i + 1), :], negb[:])
                    eng2 = nc.scalar if i % 2 == 0 else nc.sync
                    eng2.dma_start(
                        wm4r_f[32 * i:32 * (i + 1), 32 * i:32 * (i + 1)], R[:]
                    )
                wm4r = smalls.tile([128, 128], bf16, name="wm4r")
                nc.vector.tensor_copy(wm4r[:], wm4r_f[:])

          # ---- pass 2: y = x + bdiag(R) x + b4 (in place over xres) ----
          # The eviction writes y over the x chunk it just consumed (dead data),
          # so there is no staging pool whose slots are held by DMA completion;
          # output DMAs then stream 4-chunk (1 MB) groups straight from xres.
          with (
              tc.tile_pool(name="psY", bufs=6, space="PSUM") as psY,
              tc.tile_pool(name="xcast", bufs=8) as xcast,
          ):
              for c in range(nach):
                  c0 = c * 512
                  # bf16 scratch copy of the chunk feeds the correction
                  # matmul; the exact x still comes from xres in the
                  # eviction, so the identity path is exact.
                  xc = xcast.tile([128, 512], bf16, name="xc")
                  nc.scalar.copy(xc[:], xres[:, c0:c0 + 512])
                  yP = psY.tile([128, 512], f32, name="yP")
                  nc.tensor.matmul(
                      yP[:], wm4r[:], xc[:], start=True, stop=True
                  )
                  nc.vector.scalar_tensor_tensor(
                      out=xres[:, c0:c0 + 512],
                      in0=yP[:],
                      scalar=b4[:],
                      in1=xres[:, c0:c0 + 512],
                      op0=ADD, op1=ADD,
                  )
                  if (c + 1) % 4 == 0:
                      g0 = (c - 3) * 512
                      nc.sync.dma_start(
                          yout_ap[:, g0:c0 + 512], xres[:, g0:c0 + 512]
                      )
              rem = nach % 4
              if rem:
                  g0 = (nach - rem) * 512
                  nc.sync.dma_start(
                      yout_ap[:, g0:nach * 512], xres[:, g0:nach * 512]
                  )

    nc.compile()
    return nc


def _ensure_ntff_hook():
    """Register the axon NTFF profiling hook if the image's antenv lacks it.

    Only used when tracing (DBN_TRACE); mirrors what trn_boot would register
    were antenv.axon_hooks present in the image.
    """
    try:
        import antenv.axon_hooks  # noqa: F401
        return
    except ImportError:
        pass
    try:
        import types

        import antenv
        from trn_agent_boot.trn_boot import _ntff_profile_via_ctypes

        hook = _ntff_profile_via_ctypes("/opt/axon/libaxon_pjrt.so")
        mod = types.ModuleType("antenv.axon_hooks")
        mod.get_axon_ntff_profile_hook = lambda: hook
        mod.set_axon_ntff_profile_hook = lambda h: None
        sys.modules["antenv.axon_hooks"] = mod
        antenv.axon_hooks = mod
    except Exception as e:  # profiling is best-effort
        print(f"ntff hook setup failed: {e}", file=sys.stderr)


def _run(x_flat, cols, ncores, trace=False, **build_kw):
    from concourse.bass_utils import run_bass_kernel_spmd

    if trace:
        _ensure_ntff_hook()

    nc = _build(cols, ncores, **build_kw)
    in_maps = [
        {"x": np.ascontiguousarray(x_flat[ROWS * k:ROWS * (k + 1)])}
        for k in range(ncores)
    ]
    res = run_bass_kernel_spmd(
        nc, in_maps, core_ids=list(range(ncores)), trace=trace
    )
    global LAST_RESULTS
    LAST_RESULTS = res
    return np.concatenate([r["y"] for r in res.results], axis=0)


def kernel(x: np.ndarray) -> np.ndarray:
    x = np.asarray(x)
    assert x.shape == (B, C, H, W) and x.dtype == np.float32
    xf = x.reshape(B * C, COLS)
    trace = bool(os.environ.get("DBN_TRACE"))
    yf = _run(xf, COLS, NCORES, trace=trace)
    return yf.reshape(B, C, H, W)


if __name__ == "__main__":
    xs = np.load("/tmp/ref_in.npy")
    ys = kernel(xs)
    expected = np.load("/tmp/ref_out.npy")
    rel = np.linalg.norm(ys - expected) / np.linalg.norm(expected)
    print("fro_rel:", rel)
    if LAST_RESULTS is not None:
        print("exec_time_ns:", LAST_RESULTS.exec_time_ns)



# revision 3
# speedup vs baseline: 1.0852x; 1.0852x over previous
"""Decorrelated (whitening) group norm for Trainium2, 8 NeuronCores.

Problem: x (16, 64, 224, 224) f32; G=32 groups where group(channel-row r) = r % 32
(after flattening batch*channel to 1024 rows). Whitening: y = sigma^{-1/2} (x - mean)
per group, sigma the 32x32 group covariance.

v2 strategy (single NEFF, SPMD on 8 cores, data-parallel over batch):
  - Core k gets rows [128k, 128k+128) as a (128, 50176) tensor; row p is group
    p % 32. The shard is resident in SBUF as bf16 (cast once at load time on the
    ACT engine); the bf16 quantization contributes ~1.1e-3 output rel-err against
    a 2e-2 tolerance.
  - The PE HAM clock gate defaults to K=4/8 (1.2 GHz); sustained dense matmul
    activity raises it to 8/8 (2.4 GHz). A warmup burst bridges into pass 1 with
    no idle window, and a junk-matmul bridge covers the AllReduce gap, so the
    whole kernel runs warm (cold matmuls are ~2.65x slower).
  - Pass 1 (overlapped with the HBM load): per 128-col chunk, PE-transpose the
    bf16 data, DVE-evict into a (128,129) staging tile whose last column is 1.0,
    and accumulate [gram | row-sums] = T^T [T | 1] into a persistent PSUM bank.
  - Block-reduce to (32,33), AllReduce over 8 cores, then on-device 32x32 math:
    trace-normalize, 3 Newton-Schulz iterations -> wm = sigma^{-1/2}, broadcast
    bdiag(wm) via 4 tiny PE matmuls, bias b = -wm @ mean.
  - Pass 2: y = bdiag(wm) @ x_bf16 + b per 512-col chunk; evictions alternate
    DVE tensor_scalar_add and ACT Identity(bias=...) into an 8-deep f32 output
    ring that streams straight to HBM. Both passes are DMA-bound (~330 GB/s).
"""

import functools
import os
import sys

import numpy as np

if "/opt/trn_rl_repo" not in sys.path:
    sys.path.insert(0, "/opt/trn_rl_repo")

B, C, H, W = 16, 64, 224, 224
G = 32
EPS = 1e-5
NCORES = 8
ROWS = 128                 # per-core rows = 2 batches * 64 channels
COLS = H * W               # 50176
NS_ITERS = 3

LAST_RESULTS = None        # BassKernelResults of the most recent run (for test harness)


@functools.lru_cache(maxsize=4)
def _build(cols, ncores, warm_n=60, junk_n=130, cast_grain=1024, lookahead=2):
    import ml_dtypes

    import concourse.bass as bass  # noqa: F401
    import concourse.tile as tile
    from concourse import bacc, mybir

    f32 = mybir.dt.float32
    bf16 = mybir.dt.bfloat16
    ADD = mybir.AluOpType.add
    MULT = mybir.AluOpType.mult
    AFT = mybir.ActivationFunctionType

    ntch = cols // 128                # transpose chunks (392)
    nach = cols // 512                # apply chunks (98)
    ntot = 4 * cols * ncores          # elements per group, all cores

    nc = bacc.Bacc(
        "TRN2", target_bir_lowering=False, debug=False, num_devices=ncores
    )
    xin = nc.dram_tensor("x", [ROWS, cols], f32, kind="ExternalInput")
    yout = nc.dram_tensor("y", [ROWS, cols], f32, kind="ExternalOutput")
    xin_ap = xin.ap()
    yout_ap = yout.ap()

    i128_d = nc.inline_tensor(np.eye(128, dtype=np.float32), name="i128c")
    i128b_d = nc.inline_tensor(
        np.eye(128).astype(ml_dtypes.bfloat16), name="i128bc"
    )
    # e4[:, 128i:128(i+1)] is the [32,128] selector that places a 32x32 block at
    # rows/cols [32i, 32i+32) of a 128x128 block-diagonal matrix.
    e4np = np.zeros((32, 512), np.float32)
    for i in range(4):
        e4np[:, 128 * i + 32 * i: 128 * i + 32 * i + 32] = np.eye(32)
    e4_d = nc.inline_tensor(e4np, name="e4c")
    p4t_d = nc.inline_tensor(
        np.tile(np.eye(32, dtype=np.float32), (1, 4)), name="p4tc"
    )
    p4_d = nc.inline_tensor(
        np.tile(np.eye(32, dtype=np.float32), (4, 1)), name="p4c"
    )

    with tile.TileContext(nc) as tc:
        with (
            tc.tile_pool(name="consts", bufs=1) as consts,
            tc.tile_pool(name="xpool", bufs=1) as xpool,
            tc.tile_pool(name="smalls", bufs=1) as smalls,
            tc.tile_pool(name="warmp", bufs=1) as warmp,
            tc.tile_pool(name="psW", bufs=1, space="PSUM") as psW,
        ):
            # ---- constants (small DMAs, issued before the big loads) ----
            i128 = consts.tile([128, 128], f32, name="i128")
            nc.sync.dma_start(i128[:], i128_d.ap())
            i128b = consts.tile([128, 128], bf16, name="i128b")
            nc.sync.dma_start(i128b[:], i128b_d.ap())
            e4 = consts.tile([32, 512], f32, name="e4")
            nc.sync.dma_start(e4[:], e4_d.ap())
            p4t = consts.tile([32, 128], f32, name="p4t")
            nc.sync.dma_start(p4t[:], p4t_d.ap())
            p4 = consts.tile([128, 32], f32, name="p4")
            nc.sync.dma_start(p4[:], p4_d.ap())

            # ---- HAM warmup: dense same-weight matmul burst from t~0 ----
            wsrc = warmp.tile([128, 128], bf16, name="wsrc")
            nc.vector.memset(wsrc[:], 0.0)
            wps = psW.tile([128, 128], f32, name="wps")
            for i in range(warm_n):
                nc.tensor.matmul(
                    wps[:], wsrc[:], wsrc[:],
                    start=(i == 0), stop=(i == warm_n - 1),
                )

            # resident bf16 shard
            xres = xpool.tile([128, cols], bf16, name="xres")

            ones32f = consts.tile([32, 1], f32, name="ones32f")
            nc.vector.memset(ones32f[:], 1.0)
            onesrow = consts.tile([1, 32], f32, name="onesrow")
            nc.vector.memset(onesrow[:], 1.0)
            c15I = consts.tile([32, 32], f32, name="c15I")
            nc.vector.tensor_scalar_mul(c15I[:], i128[0:32, 0:32], 1.5)
            epsI = consts.tile([32, 32], f32, name="epsI")
            nc.vector.tensor_scalar_mul(epsI[:], i128[0:32, 0:32], EPS)

            # warmup AllReduce: absorbs ncfw startup latency while pass 1 runs
            with tc.tile_pool(name="dramw", bufs=1, space="DRAM") as dramw:
                win = dramw.tile([32, 1], f32, name="cc_win")
                wout = dramw.tile([32, 1], f32, name="cc_wout")
                nc.sync.dma_start(win[:], ones32f[:])
                nc.gpsimd.collective_compute(
                    "AllReduce",
                    ADD,
                    replica_groups=[list(range(ncores))],
                    ins=[win.opt()],
                    outs=[wout.opt()],
                )

            # ---- pass 1: load f32 -> cast bf16 resident -> gram | sums ----
            load_sizes = [512, 512, 1024] + [2048] * 23 + [1024]
            assert sum(load_sizes) == cols
            with (
                tc.tile_pool(name="istage", bufs=4) as istage,
                tc.tile_pool(name="psA", bufs=4, space="PSUM") as psA,
                tc.tile_pool(name="psG", bufs=1, space="PSUM") as psG,
                tc.tile_pool(name="tstage", bufs=1) as tstage,
            ):
                gramP = psG.tile([128, 129], f32, name="gramP")
                tbs = []
                for b_ in range(4):
                    tb = tstage.tile([128, 129], bf16, name=f"tb{b_}")
                    nc.vector.memset(tb[:, 128:129], 1.0)
                    tbs.append(tb)

                pos = 0
                for sz in load_sizes:
                    st = istage.tile([128, 2048], f32, name="ist")
                    nc.sync.dma_start(st[:, 0:sz], xin_ap[:, pos:pos + sz])
                    off = 0
                    while off < sz:
                        g = min(cast_grain, sz - off)
                        nc.scalar.copy(
                            xres[:, pos + off:pos + off + g],
                            st[:, off:off + g],
                        )
                        off += g
                    pos += sz

                def emit_t(j):
                    tp = psA.tile([128, 128], bf16, name="tp")
                    nc.tensor.transpose(
                        tp[:], xres[:, j * 128:(j + 1) * 128], i128b[:]
                    )
                    nc.vector.tensor_copy(tbs[j % 4][:, 0:128], tp[:])

                def emit_g(j):
                    nc.tensor.matmul(
                        gramP[:], tbs[j % 4][:, 0:128], tbs[j % 4][:, 0:129],
                        start=(j == 0), stop=(j == ntch - 1),
                    )

                la = min(lookahead, ntch)
                for j in range(ntch):
                    emit_t(j)
                    if j >= la:
                        emit_g(j - la)
                for j in range(ntch - la, ntch):
                    emit_g(j)

                gram_sb = smalls.tile([128, 129], f32, name="gram_sb")
                nc.vector.tensor_copy(gram_sb[:], gramP[:])

            # ---- block reduce to (32,33) + allreduce + 32x32 math ----
            with (
                tc.tile_pool(name="psS", bufs=3, space="PSUM") as psS,
                tc.tile_pool(name="mids", bufs=1) as mids,
            ):
                sigP = psS.tile([32, 32], f32, name="psml")
                for i in range(4):
                    nc.tensor.matmul(
                        sigP[:],
                        i128[:, 32 * i:32 * (i + 1)],
                        gram_sb[:, 32 * i:32 * (i + 1)],
                        start=(i == 0), stop=(i == 3),
                    )
                s1rP = psS.tile([32, 1], f32, name="psml")
                nc.tensor.matmul(
                    s1rP[:], p4[:], gram_sb[:, 128:129], start=True, stop=True
                )

                part = mids.tile([32, 33], f32, name="part")
                nc.vector.tensor_copy(part[:, 0:32], sigP[:])
                nc.vector.tensor_copy(part[:, 32:33], s1rP[:])

                with tc.tile_pool(name="dram", bufs=1, space="DRAM") as dram:
                    cin = dram.tile([32, 33], f32, name="cc_in")
                    cout = dram.tile([32, 33], f32, name="cc_out")
                    nc.sync.dma_start(cin[:], part[:])
                    nc.gpsimd.collective_compute(
                        "AllReduce",
                        ADD,
                        replica_groups=[list(range(ncores))],
                        ins=[cin.opt()],
                        outs=[cout.opt()],
                    )
                    ar = mids.tile([32, 33], f32, name="ar")
                    nc.sync.dma_start(ar[:], cout[:])

                # junk bridge: keeps the PE HAM-warm across the AllReduce gap
                for i in range(junk_n):
                    nc.tensor.matmul(
                        wps[:], wsrc[:], wsrc[:],
                        start=(i == 0), stop=(i == junk_n - 1),
                    )

                # ---- tiny stats math ----
                inv_n = 1.0 / float(ntot)
                mean = mids.tile([32, 1], f32, name="mean")
                nc.vector.tensor_scalar_mul(mean[:], ar[:, 32:33], inv_n)
                sig0 = mids.tile([32, 32], f32, name="sig0")
                nc.vector.tensor_scalar_mul(sig0[:], ar[:, 0:32], inv_n)

                mrowP = psS.tile([1, 32], f32, name="psml")
                nc.tensor.transpose(mrowP[:], mean[:], i128[0:32, 0:32])
                mrow = mids.tile([1, 32], f32, name="mrow")
                nc.vector.tensor_copy(mrow[:], mrowP[:])
                outerP = psS.tile([32, 32], f32, name="psml")
                nc.tensor.matmul(outerP[:], mrow[:], mrow[:], start=True, stop=True)

                sigma = mids.tile([32, 32], f32, name="sigma")
                nc.vector.scalar_tensor_tensor(
                    out=sigma[:], in0=outerP[:], scalar=-1.0, in1=sig0[:],
                    op0=MULT, op1=ADD,
                )
                nc.vector.tensor_add(sigma[:], sigma[:], epsI[:])

                # t = trace(sigma)/32 via fused diag-extract + row-reduce
                diag = mids.tile([32, 32], f32, name="diag")
                dvec = mids.tile([32, 1], f32, name="dvec")
                nc.vector.scalar_tensor_tensor(
                    out=diag[:], in0=sigma[:], scalar=1.0,
                    in1=i128[0:32, 0:32],
                    op0=MULT, op1=MULT, accum_out=dvec[:],
                )
                trP = psS.tile([1, 1], f32, name="psml")
                nc.tensor.matmul(trP[:], dvec[:], ones32f[:], start=True, stop=True)
                tr_sb = mids.tile([1, 1], f32, name="tr_sb")
                nc.vector.tensor_copy(tr_sb[:], trP[:])
                tr4P = psS.tile([32, 1], f32, name="psml")
                nc.tensor.matmul(tr4P[:], onesrow[:], tr_sb[:], start=True, stop=True)
                tA = mids.tile([32, 1], f32, name="tA")
                nc.vector.tensor_scalar_mul(tA[:], tr4P[:], 1.0 / 32.0)
                rt = mids.tile([32, 1], f32, name="rt")
                nc.vector.reciprocal(rt[:], tA[:])
                rs = mids.tile([32, 1], f32, name="rs")
                nc.scalar.activation(
                    rs[:], rt[:], AFT.Sqrt
                )

                A = mids.tile([32, 32], f32, name="A")
                nc.vector.tensor_scalar_mul(A[:], sigma[:], rt[:])

                # ---- Newton-Schulz (ping-pong, no per-iter copies) ----
                Ys = [mids.tile([32, 32], f32, name=f"Y{i}") for i in range(2)]
                Zs = [mids.tile([32, 32], f32, name=f"Z{i}") for i in range(2)]
                nc.vector.tensor_copy(Ys[0][:], A[:])
                nc.vector.tensor_copy(Zs[0][:], i128[0:32, 0:32])
                with tc.tile_pool(name="nsbuf", bufs=2) as nsbuf:
                    for it in range(NS_ITERS):
                        a, b2 = it % 2, (it + 1) % 2
                        zyP = psS.tile([32, 32], f32, name="psml")
                        nc.tensor.matmul(
                            zyP[:], Zs[a][:], Ys[a][:], start=True, stop=True
                        )
                        Wt = nsbuf.tile([32, 32], f32, name="Wt")
                        nc.vector.scalar_tensor_tensor(
                            out=Wt[:], in0=zyP[:], scalar=-0.5, in1=c15I[:],
                            op0=MULT, op1=ADD,
                        )
                        ypP = psS.tile([32, 32], f32, name="psml")
                        nc.tensor.matmul(ypP[:], Ys[a][:], Wt[:], start=True, stop=True)
                        zpP = psS.tile([32, 32], f32, name="psml")
                        nc.tensor.matmul(zpP[:], Wt[:], Zs[a][:], start=True, stop=True)
                        nc.vector.tensor_copy(Ys[b2][:], ypP[:])
                        nc.scalar.copy(Zs[b2][:], zpP[:])
                Zf = Zs[NS_ITERS % 2]

                # wm = Z / sqrt(t); b = -wm @ mean
                wm = mids.tile([32, 32], f32, name="wm")
                nc.vector.tensor_scalar_mul(wm[:], Zf[:], rs[:])
                bP = psS.tile([32, 1], f32, name="psml")
                nc.tensor.matmul(bP[:], wm[:], mean[:], start=True, stop=True)
                negb = mids.tile([32, 1], f32, name="negb")
                nc.vector.tensor_scalar_mul(negb[:], bP[:], -1.0)

                # bdiag(wm) via 4 selector matmuls; tile(negb) via 1 matmul
                wm4P = psS.tile([128, 128], f32, name="psml")
                for i in range(4):
                    nc.tensor.matmul(
                        wm4P[:, 32 * i:32 * (i + 1)],
                        e4[:, 128 * i:128 * (i + 1)],
                        wm[:],
                        start=True, stop=True,
                    )
                wm4b = smalls.tile([128, 128], bf16, name="wm4b")
                nc.vector.tensor_copy(wm4b[:], wm4P[:])
                b4P = psS.tile([128, 1], f32, name="psml")
                nc.tensor.matmul(b4P[:], p4t[:], negb[:], start=True, stop=True)
                b4 = smalls.tile([128, 1], f32, name="b4")
                nc.vector.tensor_copy(b4[:], b4P[:])

            # ---- pass 2: y = bdiag(wm) @ x_bf16 + b ----
            with (
                tc.tile_pool(name="psY", bufs=4, space="PSUM") as psY,
                tc.tile_pool(name="oring", bufs=8) as orp,
            ):
                for c in range(nach):
                    c0 = c * 512
                    yP = psY.tile([128, 512], f32, name="yP")
                    nc.tensor.matmul(
                        yP[:], wm4b[:], xres[:, c0:c0 + 512],
                        start=True, stop=True,
                    )
                    ot = orp.tile([128, 512], f32, name="ot")
                    if c % 2 == 0:
                        nc.vector.tensor_scalar_add(ot[:], yP[:], b4[:])
                    else:
                        nc.scalar.activation(
                            ot[:], yP[:], AFT.Identity, bias=b4[:], scale=1.0
                        )
                    nc.sync.dma_start(yout_ap[:, c0:c0 + 512], ot[:])

    nc.compile()
    return nc


def _ensure_ntff_hook():
    """Register the axon NTFF profiling hook if the image's antenv lacks it."""
    try:
        import antenv.axon_hooks  # noqa: F401
        return
    except ImportError:
        pass
    try:
        import types

        import antenv
        from trn_agent_boot.trn_boot import _ntff_profile_via_ctypes

        hook = _ntff_profile_via_ctypes("/opt/axon/libaxon_pjrt.so")
        mod = types.ModuleType("antenv.axon_hooks")
        mod.get_axon_ntff_profile_hook = lambda: hook
        mod.set_axon_ntff_profile_hook = lambda h: None
        sys.modules["antenv.axon_hooks"] = mod
        antenv.axon_hooks = mod
    except Exception as e:  # profiling is best-effort
        print(f"ntff hook setup failed: {e}", file=sys.stderr)


def _run(x_flat, cols, ncores, trace=False, **build_kw):
    from concourse.bass_utils import run_bass_kernel_spmd

    if trace:
        _ensure_ntff_hook()

    nc = _build(cols, ncores, **build_kw)
    in_maps = [
        {"x": np.ascontiguousarray(x_flat[ROWS * k:ROWS * (k + 1)])}
        for k in range(ncores)
    ]
    res = run_bass_kernel_spmd(
        nc, in_maps, core_ids=list(range(ncores)), trace=trace
    )
    global LAST_RESULTS
    LAST_RESULTS = res
    return np.concatenate([r["y"] for r in res.results], axis=0)


def kernel(x: np.ndarray) -> np.ndarray:
    x = np.asarray(x)
    assert x.shape == (B, C, H, W) and x.dtype == np.float32
    xf = x.reshape(B * C, COLS)
    trace = bool(os.environ.get("DBN_TRACE"))
    yf = _run(xf, COLS, NCORES, trace=trace)
    return yf.reshape(B, C, H, W)


if __name__ == "__main__":
    xs = np.load("/tmp/ref_in.npy")
    ys = kernel(xs)
    expected = np.load("/tmp/ref_out.npy")
    rel = np.linalg.norm(ys - expected) / np.linalg.norm(expected)
    print("fro_rel:", rel)
    if LAST_RESULTS is not None:
        print("exec_time_ns:", LAST_RESULTS.exec_time_ns)
